# revision 6
# baseline (speedup 1.0000x reference)
"""3-layer GAT on 8 TRN2 NeuronCores — wire-optimized + For_i hardware loops.

Cost model of this environment (axon-tunneled devices):
  - H2D bytes dominate (~30-70MB/s tunnel): ship minimal bytes (bf16 x/W,
    compact 16-row gather indices, bf16 slot metadata, single-row biases).
  - Per STATIC instruction overhead ~100us/instr: all per-batch/per-tile
    loops are For_i hardware loops (static program ~300 instrs).

Design: nodes partitioned by dst across 8 cores. Edges bucketed into fixed
128-dst windows (window b = dst in [b*128,(b+1)*128)); every node is a dst
(self-loops) so windows tile the shard exactly. Per window: gather fat rows
[h|asrc] of src nodes from an AllGather'ed per-layer table (low/high split
for int16 index reach), gather per-edge adst from a core-local narrow
table, softmax via exp(leakyrelu()) with segment-sum by one-hot matmul in
PSUM (extra payload column = denominator), epilogue writes rows
[b*128,(b+1)*128) with a plain DMA.
"""
import os
import numpy as np
import ml_dtypes

import jax

# Persistent XLA compilation cache: repeat calls (and fresh processes) skip
# the ~0.3s per-call bir_verify/neuronx recompile of the wrapper executable.
try:
    jax.config.update("jax_compilation_cache_dir", "/tmp/jax_comp_cache")
    jax.config.update("jax_persistent_cache_min_entry_size_bytes", 0)
    jax.config.update("jax_persistent_cache_min_compile_time_secs", 0)
except Exception:
    pass

from concourse import bass, bacc, mybir, tile
from concourse.bass import ds, ts
from concourse.bass_utils import run_bass_kernel_spmd

f32 = mybir.dt.float32
bf16 = mybir.dt.bfloat16
i16 = mybir.dt.int16
i8 = mybir.dt.int8
i32 = mybir.dt.int32
Alu = mybir.AluOpType
Act = mybir.ActivationFunctionType

FULL_CFG = dict(
    N=50000, IN=128, HID=64, OUT=64, NH=4, E=800000, R=8,
    VSPLIT=32768,                 # int16 index reach for the fat gather
)


def make_cfg(**over):
    cfg = dict(FULL_CFG)
    cfg.update(over)
    N, R = cfg["N"], cfg["R"]
    assert N % R == 0
    cfg["SHARD"] = N // R
    # local rows: shard + >=2 pad rows, multiple of 128
    cfg["LPAD"] = ((cfg["SHARD"] + 2 + 127) // 128) * 128
    cfg["TROWS"] = R * cfg["LPAD"]
    cfg["PADROW"] = cfg["SHARD"]  # local pad row (asrc=-1e30 in every shard)
    cfg["B"] = (cfg["SHARD"] + 127) // 128
    assert cfg["B"] * 128 <= cfg["LPAD"]
    if cfg["TROWS"] <= cfg["VSPLIT"]:
        cfg["VSPLIT"] = cfg["TROWS"]
    else:
        assert cfg["TROWS"] - cfg["VSPLIT"] <= 32768
        r = 0
        while r * cfg["LPAD"] + cfg["SHARD"] < cfg["VSPLIT"]:
            r += 1
        cfg["PADROW_H"] = r * cfg["LPAD"] + cfg["SHARD"]
        assert cfg["PADROW_H"] >= cfg["VSPLIT"]
    cfg["FATW12"] = 320   # h(256) | asrc(4) | pad   (f32, 256B multiple)
    cfg["FATW3"] = 128    # h(64) | asrc(1) | pad
    cfg["NRW"] = 8        # adst table row width (plain DMA per window)
    return cfg


def _renum(n, cfg):
    return (n // cfg["SHARD"]) * cfg["LPAD"] + (n % cfg["SHARD"])


def _wrap_idx(idx_flat):
    """dma_gather int16 index layout: ordinal i at [i%16, i//16] (16-row wrap)."""
    n = len(idx_flat)
    assert n % 16 == 0
    return np.asarray(idx_flat, np.int16).reshape(n // 16, 16).T


def _core_windows(src_g, dst_l, cfg):
    """Sort one core's edges by dst and slice into fixed 128-dst windows.

    Returns list of (src_global, slot) per window plus max low/high counts.
    """
    VS = cfg["VSPLIT"]
    order = np.argsort(dst_l, kind="stable")
    s = src_g[order]
    d = dst_l[order]
    wins = []
    maxL = maxH = 0
    for b in range(cfg["B"]):
        lo = np.searchsorted(d, b * 128, "left")
        hi = np.searchsorted(d, (b + 1) * 128, "left")
        se = s[lo:hi]
        slot = d[lo:hi] - b * 128
        lowm = se < VS
        eL, sL = se[lowm], slot[lowm]
        eH, sH = se[~lowm], slot[~lowm]
        maxL = max(maxL, len(eL))
        maxH = max(maxH, len(eH))
        wins.append((eL, sL, eH, sH))
    return wins, maxL, maxH


def _fill_core(wins, cfg):
    """Build idxc [16, B*CW] i16 / mfb [B*128, T] bf16 for one core."""
    B, TL, TH, T, CW = cfg["B"], cfg["TL"], cfg["TH"], cfg["T"], cfg["CW"]
    VS = cfg["VSPLIT"]
    PAD_L = cfg["PADROW"]
    PAD_H = max(cfg.get("PADROW_H", 0) - VS, 0)
    idxc = np.zeros((B, 16, CW), np.int16)
    mfb = np.zeros((B, 128, T), np.float32)
    for b, (eL, sL, eH, sH) in enumerate(wins):
        iL = np.full(TL * 128, PAD_L, np.int64)
        iL[:len(eL)] = eL
        iH = np.full(TH * 128, PAD_H, np.int64)
        iH[:len(eH)] = eH - VS
        slots = np.zeros(T * 128, np.int64)
        slots[:len(sL)] = sL
        slots[TL * 128:TL * 128 + len(sH)] = sH
        c = 0
        idxc[b, :, c:c + TL * 8] = _wrap_idx(iL); c += TL * 8
        if TH:
            idxc[b, :, c:c + TH * 8] = _wrap_idx(iH); c += TH * 8
        mfb[b] = slots.reshape(T, 128).T
    idxc = np.ascontiguousarray(idxc.transpose(1, 0, 2).reshape(16, B * CW))
    return idxc, mfb.reshape(B * 128, T).astype(np.int8)


def prep_host(x, edge_index, cfg):
    """Host-side sharding prep. Returns (per_core_inputs, cfg w/ TL/TH)."""
    N, R, SHARD, LPAD = cfg["N"], cfg["R"], cfg["SHARD"], cfg["LPAD"]
    IN = cfg["IN"]
    src = np.concatenate([np.asarray(edge_index[0]), np.arange(N)]).astype(np.int64)
    dst = np.concatenate([np.asarray(edge_index[1]), np.arange(N)]).astype(np.int64)
    src_g = _renum(src, cfg)

    allwins = []
    maxL = maxH = 0
    for r in range(R):
        m = (dst // SHARD) == r
        wins, mL, mH = _core_windows(src_g[m], dst[m] - r * SHARD, cfg)
        allwins.append(wins)
        maxL, maxH = max(maxL, mL), max(maxH, mH)
    cfg["TL"] = (maxL + 127) // 128
    cfg["TH"] = max((maxH + 127) // 128, 1)
    cfg["T"] = cfg["TL"] + cfg["TH"]
    cfg["CW"] = (cfg["TL"] + cfg["TH"]) * 8

    # int8 x with per-feature scales (folded into W1 rows on the host)
    xs = np.maximum(np.abs(x).max(axis=0) / 127.0, 1e-12)
    xq = np.round(x / xs).astype(np.int8)
    per_core = []
    for r in range(R):
        idxc, mfb = _fill_core(allwins[r], cfg)
        xm = np.zeros((IN, LPAD), np.int8)
        xm[:, :SHARD] = xq[r * SHARD:(r + 1) * SHARD].T
        per_core.append(dict(idxc=idxc, mfb=mfb, xmine=xm))
    return per_core, xs


def _aug_w(W, a_s, a_d, nh, hid, row_scale=None):
    """[inF, outF+2*nh] = [W.T | As | Ad] (rows optionally pre-scaled)."""
    inf = W.shape[1]
    Wr = W.reshape(nh, hid, inf)
    As = np.einsum("hci,hc->ih", Wr, a_s)
    Ad = np.einsum("hci,hc->ih", Wr, a_d)
    full = np.concatenate([W.T, As, Ad], axis=1)
    if row_scale is not None:
        full = full * row_scale[:, None]
    return full.astype(ml_dtypes.bfloat16)


def build_nc(cfg):
    R = cfg["R"]
    LPAD, TROWS, SHARD = cfg["LPAD"], cfg["TROWS"], cfg["SHARD"]
    B, T, TL, TH = cfg["B"], cfg["T"], cfg["TL"], cfg["TH"]
    VS = cfg["VSPLIT"]
    NH, HID, OUT, IN = cfg["NH"], cfg["HID"], cfg["OUT"], cfg["IN"]
    F = NH * HID              # 256
    FATW, FATW3, NRW = cfg["FATW12"], cfg["FATW3"], cfg["NRW"]
    CW = cfg["CW"]
    NLT = LPAD // 128
    BIASW = 2 * F + OUT

    nc = bacc.Bacc("TRN2", target_bir_lowering=False, debug=False, num_devices=R)

    P = {}
    P["xmine"] = nc.declare_dram_parameter("xmine", [IN, LPAD], i8, isOutput=False)
    P["w1t"] = nc.declare_dram_parameter("w1t", [IN, F + 2 * NH], bf16, isOutput=False)
    P["w2t"] = nc.declare_dram_parameter("w2t", [F, F + 2 * NH], bf16, isOutput=False)
    P["w3t"] = nc.declare_dram_parameter("w3t", [F, OUT + 2], bf16, isOutput=False)
    P["bcat"] = nc.declare_dram_parameter("bcat", [1, BIASW], f32, isOutput=False)
    P["idxc"] = nc.declare_dram_parameter("idxc", [16, B * CW], i16, isOutput=False)
    P["mfb"] = nc.declare_dram_parameter("mfb", [B * 128, T], i8, isOutput=False)
    out_p = nc.declare_dram_parameter("out", [B * 128, OUT + 4], i8, isOutput=True)

    own_h1 = nc.dram_tensor("own_h1", [LPAD, FATW], f32)
    tbl1 = nc.dram_tensor("tbl1", [TROWS, FATW], f32, addr_space="Shared")
    tbl2 = nc.dram_tensor("tbl2", [TROWS, FATW], f32, addr_space="Shared")
    tbl3 = nc.dram_tensor("tbl3", [TROWS, FATW3], f32, addr_space="Shared")
    own_h2 = nc.dram_tensor("own_h2", [LPAD, FATW], f32)
    own_h3 = nc.dram_tensor("own_h3", [LPAD, FATW3], f32)
    adl1 = nc.dram_tensor("adl1", [LPAD, NRW], f32)
    adl2 = nc.dram_tensor("adl2", [LPAD, NRW], f32)
    adl3 = nc.dram_tensor("adl3", [LPAD, NRW], f32)
    own_x1 = nc.dram_tensor("own_x1", [LPAD, F], f32)
    own_x2 = nc.dram_tensor("own_x2", [LPAD, F], f32)
    ridxd = nc.dram_tensor("ridxd", [B * 128, CW], i16)

    with tile.TileContext(nc) as tc:
        with tc.tile_pool(name="const", bufs=1) as cpool, \
             tc.tile_pool(name="work", bufs=3) as wpool, \
             tc.tile_pool(name="gath", bufs=2) as gpool, \
             tc.tile_pool(name="psA", bufs=2, space="PSUM") as psA, \
             tc.tile_pool(name="psB", bufs=2, space="PSUM") as psB, \
             tc.tile_pool(name="psC", bufs=2, space="PSUM") as psC:

            w1t = cpool.tile([IN, F + 2 * NH], bf16, tag="w1t")
            nc.sync.dma_start(out=w1t[:], in_=P["w1t"][:])
            w2t_lo = cpool.tile([128, F + 2 * NH], bf16, tag="w2lo")
            nc.sync.dma_start(out=w2t_lo[:], in_=P["w2t"][0:128, :])
            w2t_hi = cpool.tile([128, F + 2 * NH], bf16, tag="w2hi")
            nc.sync.dma_start(out=w2t_hi[:], in_=P["w2t"][128:256, :])
            w3t_lo = cpool.tile([128, OUT + 2], bf16, tag="w3lo")
            nc.sync.dma_start(out=w3t_lo[:], in_=P["w3t"][0:128, :])
            w3t_hi = cpool.tile([128, OUT + 2], bf16, tag="w3hi")
            nc.sync.dma_start(out=w3t_hi[:], in_=P["w3t"][128:256, :])

            # biases: [1, BIASW] -> [128, BIASW] by log-doubling partition DMAs
            bcat = cpool.tile([128, BIASW], f32, tag="bcat")
            nc.sync.dma_start(out=bcat[0:1, :], in_=P["bcat"][:])
            k = 1
            while k < 128:
                nc.sync.dma_start(out=bcat[k:2 * k, :], in_=bcat[0:k, :])
                k *= 2
            b1 = bcat[:, 0:F]
            b2 = bcat[:, F:2 * F]
            b3 = bcat[:, 2 * F:2 * F + OUT]

            # iof[p, j] = j ; ident = eye(128)
            iof = cpool.tile([128, 128], f32, tag="iof")
            nc.gpsimd.iota(iof[:], pattern=[[1, 128]], base=0,
                           channel_multiplier=0,
                           allow_small_or_imprecise_dtypes=True)
            ones_sq = cpool.tile([128, 128], f32, tag="ones_sq")
            nc.vector.memset(ones_sq[:], 1.0)
            ident = cpool.tile([128, 128], f32, tag="ident")
            nc.gpsimd.affine_select(ident[:], ones_sq[:], pattern=[[-1, 128]],
                                    base=0, channel_multiplier=1,
                                    compare_op=Alu.is_equal, fill=0.0)

            zero = cpool.tile([128, F], f32, tag="zero")
            nc.vector.memset(zero[:], 0.0)
            neg = cpool.tile([128, NH], f32, tag="neg")
            nc.vector.memset(neg[:], -1e30)

            # replicate compact idx [16, B*CW] -> ridxd [B*128, CW] (DRAM)
            rv = ridxd[:].rearrange("(b g p) c -> g p b c", g=8, p=16)
            iv = P["idxc"][:].rearrange("p (b c) -> p b c", c=CW)
            for k in range(8):
                nc.sync.dma_start(out=rv[k], in_=iv[:])

            # ---------------- L1 dense: own shard [h1|asrc1] + adl1 ----------
            with tc.For_i(0, NLT) as t:
                xc = wpool.tile([IN, 128], i8, tag="xc")
                nc.sync.dma_start(out=xc[:], in_=P["xmine"][:, ts(t, 128)])
                xcb = wpool.tile([IN, 128], bf16, tag="xcb")
                nc.scalar.activation(xcb[:], xc[:], Act.Copy)
                ps = psA.tile([128, F + 2 * NH], f32, tag="dens")
                nc.tensor.matmul(ps[:], lhsT=xcb[:], rhs=w1t[:], start=True, stop=True)
                hrow = wpool.tile([128, FATW], f32, tag="hrow")
                nc.vector.tensor_copy(hrow[:, :F + NH], ps[:, :F + NH])
                nc.vector.memset(hrow[:, F + NH:], 0.0)
                nc.sync.dma_start(out=own_h1[ts(t, 128), :], in_=hrow[:])
                ad = wpool.tile([128, NRW], f32, tag="ad")
                nc.scalar.activation(ad[:, 0:NH], ps[:, F + NH:F + 2 * NH], Act.Copy)
                nc.vector.memset(ad[:, NH:], 0.0)
                nc.sync.dma_start(out=adl1[ts(t, 128), :], in_=ad[:])
            npad = LPAD - SHARD
            nc.sync.dma_start(out=own_h1[SHARD:LPAD, F:F + NH], in_=neg[:npad, :])

            # ---------------- generic agg layer (For_i over windows) ---------
            def agg_layer(tbl, adl, fatw, nh, c, payw, bias, relu, out_dram, outw,
                          out_dt, quant=None):
                # payw = nh*c + nh ; outw = nh*c
                with tc.For_i(0, B) as b:
                    ix = wpool.tile([128, CW], i16, tag="ix")
                    nc.sync.dma_start(out=ix[:], in_=ridxd[ts(b, 128), :])
                    mfb_t = wpool.tile([128, T], i8, tag="mfb")
                    nc.sync.dma_start(out=mfb_t[:], in_=P["mfb"][ts(b, 128), :])
                    mf = wpool.tile([128, T], f32, tag="mf")
                    nc.scalar.activation(mf[:], mfb_t[:], Act.Copy)

                    gat = gpool.tile([128, T * fatw], f32, tag="gat")
                    g3 = gat[:].rearrange("p (t q) -> p t q", q=fatw)
                    SP = False
                    nc.gpsimd.dma_gather(
                        g3[:, 0:TL, :], tbl[0:VS, :], ix[:, 0:TL * 8],
                        TL * 128, TL * 128, fatw, single_packet=SP)
                    if TH:
                        nc.gpsimd.dma_gather(
                            g3[:, TL:T, :], tbl[VS:TROWS, :],
                            ix[:, TL * 8:(TL + TH) * 8],
                            TH * 128, TH * 128, fatw, single_packet=SP)
                    adT = wpool.tile([128, NRW], f32, tag="adT")
                    nc.sync.dma_start(out=adT[:], in_=adl[ts(b, 128), :])

                    S = gpool.tile([128, T * 128], f32, tag="S")
                    nc.vector.tensor_tensor(
                        S[:].rearrange("p (t d) -> p t d", d=128),
                        iof[:].unsqueeze(1).to_broadcast([128, T, 128]),
                        mf[:].unsqueeze(2).to_broadcast([128, T, 128]),
                        Alu.is_equal)

                    # per-edge adst: adE[p, t, h] = adT[slot(p,t), h] via
                    # S_t^T (transpose) one-hot matmuls
                    adEs = wpool.tile([128, T * nh], f32, tag="adEs")
                    for t in range(T):
                        ptr = psB.tile([128, 128], f32, tag="tr")
                        nc.tensor.transpose(out=ptr[:],
                                            in_=S[:, t * 128:(t + 1) * 128],
                                            identity=ident[:])
                        stp = wpool.tile([128, 128], f32, tag="stp")
                        nc.scalar.activation(stp[:], ptr[:], Act.Copy)
                        pae = psB.tile([128, nh], f32, tag="adE")
                        nc.tensor.matmul(pae[:], lhsT=stp[:], rhs=adT[:, 0:nh],
                                         start=True, stop=True)
                        nc.vector.tensor_copy(adEs[:, t * nh:(t + 1) * nh], pae[:])
                    lg = wpool.tile([128, T * nh], f32, tag="lg")
                    nc.vector.tensor_tensor(
                        lg[:].rearrange("p (t h) -> p t h", h=nh),
                        g3[:, :, nh * c:nh * c + nh],
                        adEs[:].rearrange("p (t h) -> p t h", h=nh), Alu.add)
                    lg2 = wpool.tile([128, T * nh], f32, tag="lg2")
                    nc.vector.tensor_scalar(lg2[:], lg[:], 0.2, None, Alu.mult)
                    lmax = wpool.tile([128, T * nh], f32, tag="lmax")
                    nc.vector.tensor_tensor(lmax[:], lg[:], lg2[:], Alu.max)

                    pay = gpool.tile([128, T * payw], f32, tag="pay")
                    p3 = pay[:].rearrange("p (t q) -> p t q", q=payw)
                    nc.scalar.activation(
                        p3[:, :, nh * c:nh * c + nh],
                        lmax[:].rearrange("p (t h) -> p t h", h=nh), Act.Exp)
                    nc.vector.tensor_tensor(
                        p3[:, :, 0:nh * c].rearrange("p t (h q) -> p t h q", q=c),
                        g3[:, :, 0:nh * c].rearrange("p t (h q) -> p t h q", q=c),
                        p3[:, :, nh * c:nh * c + nh].unsqueeze(3).to_broadcast(
                            [128, T, nh, c]),
                        Alu.mult)

                    ps = psC.tile([128, payw], f32, tag="agg")
                    for t in range(T):
                        nc.tensor.matmul(
                            ps[:], lhsT=S[:, t * 128:(t + 1) * 128],
                            rhs=pay[:, t * payw:(t + 1) * payw],
                            start=(t == 0), stop=(t == T - 1))

                    den = wpool.tile([128, nh], f32, tag="den")
                    nc.vector.tensor_scalar(den[:], ps[:, nh * c:nh * c + nh],
                                            1e-16, None, Alu.add)
                    rden = wpool.tile([128, nh], f32, tag="rden")
                    nc.vector.reciprocal(rden[:], den[:])
                    orow = wpool.tile([128, outw], f32, tag="orow")
                    nc.vector.tensor_tensor(
                        orow[:].rearrange("p (h q) -> p h q", q=c),
                        ps[:, 0:nh * c].rearrange("p (h q) -> p h q", q=c),
                        rden[:].unsqueeze(2).to_broadcast([128, nh, c]),
                        Alu.mult)
                    ob = wpool.tile([128, outw], f32, tag="ob")
                    nc.vector.tensor_tensor(ob[:], orow[:], bias[:, :outw], Alu.add)
                    if quant is None:
                        ofin = wpool.tile([128, outw], out_dt, tag="ofin")
                        if relu:
                            nc.scalar.activation(ofin[:], ob[:], Act.Relu)
                        else:
                            nc.scalar.activation(ofin[:], ob[:], Act.Copy)
                        nc.sync.dma_start(out=out_dram[ts(b, 128), 0:outw],
                                          in_=ofin[:])
                    else:
                        # int8 rows scaled by rowwise absmax/127, scales in
                        # quant[:, 0]; log-tree max over column halves
                        oabs = wpool.tile([128, outw], f32, tag="oabs")
                        nc.scalar.activation(oabs[:], ob[:], Act.Abs)
                        w = outw
                        while w > 1:
                            h2 = w // 2
                            nc.vector.tensor_tensor(
                                oabs[:, 0:h2], oabs[:, 0:h2], oabs[:, h2:w],
                                Alu.max)
                            w = h2
                        rsc = wpool.tile([128, 8], f32, tag="rsc")
                        nc.vector.tensor_scalar(rsc[:, 0:1], oabs[:, 0:1],
                                                1.0 / 127.0, 1e-30,
                                                Alu.mult, Alu.add)
                        rinv = wpool.tile([128, 1], f32, tag="rinv")
                        nc.vector.reciprocal(rinv[:], rsc[:, 0:1])
                        qf = wpool.tile([128, outw], f32, tag="qf")
                        nc.vector.tensor_tensor(
                            qf[:], ob[:],
                            rinv[:].to_broadcast([128, outw]), Alu.mult)
                        of2 = wpool.tile([128, outw + 4], i8, tag="of2")
                        nc.scalar.activation(of2[:, 0:outw], qf[:], Act.Copy)
                        nc.vector.tensor_copy(of2[:, outw:outw + 4],
                                              rsc[:, 0:1].bitcast(i8))
                        nc.sync.dma_start(out=out_dram[ts(b, 128), :], in_=of2[:])

            # ---------------- own-shard dense (L2/L3, For_i over tiles) ------
            def dense_own(x_dram, wlo, whi, own_h, adl, asrc_cols):
                with tc.For_i(0, NLT) as t:
                    xr = wpool.tile([128, F], f32, tag="xr")
                    nc.sync.dma_start(out=xr[:], in_=x_dram[ts(t, 128), :])
                    pt0 = psB.tile([128, 128], f32, tag="tr")
                    nc.tensor.transpose(out=pt0[:], in_=xr[:, 0:128], identity=ident[:])
                    xT0 = wpool.tile([128, 128], bf16, tag="xT0")
                    nc.scalar.activation(xT0[:], pt0[:], Act.Copy)
                    pt1 = psB.tile([128, 128], f32, tag="tr")
                    nc.tensor.transpose(out=pt1[:], in_=xr[:, 128:256], identity=ident[:])
                    xT1 = wpool.tile([128, 128], bf16, tag="xT1")
                    nc.scalar.activation(xT1[:], pt1[:], Act.Copy)
                    nw = wlo.shape[1]
                    ps = psA.tile([128, nw], f32, tag="dens")
                    nc.tensor.matmul(ps[:], lhsT=xT0[:], rhs=wlo[:], start=True, stop=False)
                    nc.tensor.matmul(ps[:], lhsT=xT1[:], rhs=whi[:], start=False, stop=True)
                    nasrc = asrc_cols
                    hw_ = nw - 2 * nasrc
                    fatw_ = own_h.shape[1]
                    hrow = wpool.tile([128, fatw_], f32, tag="hrow2")
                    nc.vector.tensor_copy(hrow[:, :hw_ + nasrc], ps[:, :hw_ + nasrc])
                    nc.vector.memset(hrow[:, hw_ + nasrc:], 0.0)
                    nc.sync.dma_start(out=own_h[ts(t, 128), :], in_=hrow[:])
                    ad = wpool.tile([128, NRW], f32, tag="ad")
                    nc.scalar.activation(ad[:, 0:nasrc],
                                         ps[:, hw_ + nasrc:hw_ + 2 * nasrc], Act.Copy)
                    nc.vector.memset(ad[:, nasrc:], 0.0)
                    nc.sync.dma_start(out=adl[ts(t, 128), :], in_=ad[:])

            # ================= pipeline =================
            PHASE = int(os.environ.get("GAT_PHASE", "9"))

            def allgather(own, tbl):
                if os.environ.get("GAT_SKIP_CC"):
                    nc.sync.dma_start(out=tbl[0:LPAD, :], in_=own[:])
                else:
                    nc.gpsimd.collective_compute(
                        "AllGather", Alu.bypass, replica_groups=[list(range(R))],
                        ins=[own[:].opt()], outs=[tbl[:].opt()])

            if PHASE >= 1:
                allgather(own_h1, tbl1)

            ZPAD = LPAD - B * 128   # own_x rows beyond the windows
            if PHASE >= 2:
                if ZPAD:
                    nc.sync.dma_start(out=own_x1[B * 128:LPAD, :], in_=zero[:ZPAD, :])
                agg_layer(tbl1, adl1, FATW, NH, HID, F + NH, b1, True, own_x1, F, f32)

            if PHASE >= 3:
                dense_own(own_x1, w2t_lo, w2t_hi, own_h2, adl2, NH)
                nc.sync.dma_start(out=own_h2[SHARD:LPAD, F:F + NH], in_=neg[:npad, :])
            if PHASE >= 4:
                allgather(own_h2, tbl2)

            if PHASE >= 5:
                if ZPAD:
                    nc.sync.dma_start(out=own_x2[B * 128:LPAD, :], in_=zero[:ZPAD, :])
                agg_layer(tbl2, adl2, FATW, NH, HID, F + NH, b2, True, own_x2, F, f32)

            if PHASE >= 6:
                dense_own(own_x2, w3t_lo, w3t_hi, own_h3, adl3, 1)
                nc.sync.dma_start(out=own_h3[SHARD:LPAD, OUT:OUT + 1],
                                  in_=neg[:npad, 0:1])
            if PHASE >= 7:
                allgather(own_h3, tbl3)

            if PHASE >= 8:
                agg_layer(tbl3, adl3, FATW3, 1, OUT, OUT + 1, b3, False, out_p, OUT,
                          i8, quant=True)
            if PHASE < 8:
                zo = wpool.tile([128, OUT + 4], i8, tag="zo")
                nc.vector.memset(zo[:], 0.0)
                nc.sync.dma_start(out=out_p[0:128, :], in_=zo[:])

    if not nc.is_finalized():
        nc.finalize()
    return nc


def make_inputs(inputs, cfg):
    """Host prep: returns (nc-ready in_maps list, cfg with TL/TH/T/CW set)."""
    x = np.asarray(inputs["x"], np.float32)
    edge_index = np.asarray(inputs["edge_index"])
    NH, HID, OUT = cfg["NH"], cfg["HID"], cfg["OUT"]
    per_core, xs = prep_host(x, edge_index, cfg)

    w1t = _aug_w(np.asarray(inputs["W1"], np.float32),
                 np.asarray(inputs["as1"], np.float32),
                 np.asarray(inputs["ad1"], np.float32), NH, HID, row_scale=xs)
    w2t = _aug_w(np.asarray(inputs["W2"], np.float32),
                 np.asarray(inputs["as2"], np.float32),
                 np.asarray(inputs["ad2"], np.float32), NH, HID)
    w3t = _aug_w(np.asarray(inputs["W3"], np.float32),
                 np.asarray(inputs["as3"], np.float32),
                 np.asarray(inputs["ad3"], np.float32), 1, OUT)
    F = NH * HID
    bcat = np.concatenate([
        np.asarray(inputs["b1"], np.float32).ravel(),
        np.asarray(inputs["b2"], np.float32).ravel(),
        np.asarray(inputs["b3"], np.float32).ravel()]).reshape(1, 2 * F + OUT)

    shared = dict(w1t=w1t, w2t=w2t, w3t=w3t, bcat=bcat)
    in_maps = []
    for r in range(cfg["R"]):
        m = dict(shared)
        m["idxc"] = per_core[r]["idxc"]
        m["mfb"] = per_core[r]["mfb"]
        m["xmine"] = per_core[r]["xmine"]
        in_maps.append(m)
    return in_maps


_KERNEL_CACHE = {}


def run(inputs, cfg=None, trace=False):
    cfg = cfg or make_cfg()
    in_maps = make_inputs(inputs, cfg)
    key = (cfg["N"], cfg["E"], cfg["B"], cfg["TL"], cfg["TH"])
    if key not in _KERNEL_CACHE:
        _KERNEL_CACHE[key] = build_nc(cfg)
    nc = _KERNEL_CACHE[key]
    try:
        res = run_bass_kernel_spmd(nc, in_maps, list(range(cfg["R"])), trace=trace)
    except Exception:
        # transient device-unrecoverable right after another process released
        # the cores; one backoff retry
        import time
        time.sleep(30)
        res = run_bass_kernel_spmd(nc, in_maps, list(range(cfg["R"])), trace=trace)
    OUT = cfg["OUT"]
    outs = []
    for r in range(cfg["R"]):
        raw = np.asarray(res.results[r]["out"][:cfg["SHARD"]])
        q = raw[:, 0:OUT].astype(np.float32)
        s = np.ascontiguousarray(raw[:, OUT:OUT + 4]).view(np.float32)
        outs.append(q * s)
    return np.concatenate(outs, axis=0), res


def kernel(**inputs):
    out, _ = run(inputs)
    return out.astype(np.float32)
